# revision 47
# baseline (speedup 1.0000x reference)
"""Cross-attention decode kernel for Trainium2 (8 NeuronCores, Bass/Tile).

Reference computation (B=256, N=32768, D=1024, H=16, DH=64):
    qh = (q @ W_q.T)   [B,H,DH]
    kh = (k @ W_k.T)   [N,H,DH]
    vh = (v @ W_v.T)   [N,H,DH]
    score = einsum('bhd,nhd->hbn', qh, kh) / sqrt(DH)
    out   = einsum('hbn,nhd->bhd', softmax(score, -1), vh)  -> [B, D]

Sharding: split N across the 8 cores (flash-decoding style split-K).  Each
core projects its k/v shard, computes unnormalized exp-scores (no max
subtraction needed: scores ~ N(0,1), max < ~7, exp is safe in fp32), and
accumulates per-head numerator sum_n p*vh plus denominator sum_n p (the
denominator is obtained for free by appending a ones-column to vh in the
context matmul).  The host adds the 8 partial (num, den) pairs and divides.

Layout trick: every matmul contracts on the partition dim, so all operands
are staged pre-transposed from the host (kT, vT, W*T, qT).  Scores are
produced transposed [keys, b] so the context matmul needs no transposes
anywhere on the device.
"""

import sys

for _p in ("/opt/trn_rl_repo",):
    if _p not in sys.path:
        sys.path.insert(0, _p)

import numpy as np

B, N, D, H = 256, 32768, 1024, 16
DH = D // H            # 64
NCORES = 8
NS = N // NCORES       # 4096 keys per core
SBK = 512              # keys per super-block
NSB = NS // SBK        # 8
KC = 128               # key chunk (scores/ctx granularity)
NKC = SBK // KC        # 4
DC = 128               # contraction chunk
NDC = D // DC          # 8
HG = 4                 # heads per scores-psum group
NHG = H // HG          # 4

_F16 = np.float16

_CACHED = {}


def _build():
    import concourse.mybir as mybir
    from concourse import bacc
    from concourse.tile import TileContext

    f16 = mybir.dt.float16
    f32 = mybir.dt.float32

    # Bacc (not raw Bass): its finalize() runs generate_event_semaphores,
    # which splits multi-sem waits into single-wait form (TRN2 ISA allows
    # one wait per instruction) — walrus rejects the IR otherwise.
    nc = bacc.Bacc()

    # host-swizzled layouts: qT/wkT/wvT are [128, c, ...] partition-major so
    # each DMA is fully contiguous per partition; wqT additionally has the
    # m-chunk outermost so the prologue can stream it in 8 small DMAs.
    qT = nc.declare_dram_parameter("qT", [128, NDC * B], f16, isOutput=False)
    wqT = nc.declare_dram_parameter("wqT", [NDC, 128, NDC * DC], f16, isOutput=False)
    wkT = nc.declare_dram_parameter("wkT", [128, NDC * D], f16, isOutput=False)
    wvT = nc.declare_dram_parameter("wvT", [128, NDC * D], f16, isOutput=False)
    # kT/vT are host-swizzled to [128, sb, c, keys] so one super-block's
    # load is a SINGLE dma with an 8KB-contiguous run per partition (128
    # descriptors instead of 4 calls x 256 1KB-descriptors)
    kT = nc.declare_dram_parameter("kT", [128, NSB * NDC * SBK], f16, isOutput=False)
    vT = nc.declare_dram_parameter("vT", [128, NSB * NDC * SBK], f16, isOutput=False)
    out = nc.declare_dram_parameter("out", [DH + 1, H, B], f32, isOutput=True)

    Exp = mybir.ActivationFunctionType.Exp

    with TileContext(nc) as tc:
        with (
            tc.tile_pool(name="wk", bufs=1) as wk_pool,
            tc.tile_pool(name="wv", bufs=1) as wv_pool,
            tc.tile_pool(name="qh", bufs=1) as qh_pool,
            tc.tile_pool(name="cs", bufs=1) as cs_pool,
        ):
            # qh^T resident: [dout(part), dout_chunk, b]
            qh_sb = qh_pool.tile([128, NDC, B], f16)
            # numerator/denominator accumulator: [dh+1, h, b].  No memset:
            # the sb==0 iteration writes (tensor_copy) instead of adding, so
            # nothing blocks the warm-up matmuls at kernel start.
            ctx_sb = cs_pool.tile([DH + 1, H, B], f32)

            # ---- prologue: qh^T = (q @ Wq.T)^T ----
            # Issue the q-side DMAs FIRST: HWDGE DMAs execute FIFO per
            # engine, so putting the big weight loads first would delay the
            # first matmul by ~20us.
            wk_sb = wk_pool.tile([128, NDC, D], f16)
            wv_sb = wv_pool.tile([128, NDC, D], f16)
            # qt/wq pools stay open for the whole kernel: releasing them lets
            # the kv pool reuse their SBUF range, which adds a false WAR dep
            # that stalls the first kt/vt DMAs ~10us behind the prologue.
            qt_pool = tc.alloc_tile_pool(name="qt", bufs=1)
            wq_pool = tc.alloc_tile_pool(name="wq", bufs=2)
            # kv pool opened before the prologue so sb0's kt DMA can be
            # issued early (between wk and wq) on the sync queue
            kv_pool = tc.alloc_tile_pool(name="kv", bufs=2)
            with (
                tc.tile_pool(name="pq", bufs=4, space="PSUM") as pq_pool,
            ):
                # PE warm-up: dummy matmuls during the initial DMA wait so the
                # HAM clock gate reaches 8/8 before the real pipeline starts.
                warm = qt_pool.tile([128, 512], f16, name="warm", tag="warm")
                nc.gpsimd.memset(warm, 0.0)
                # dummy activation first: pulls the ~2.7us exp_and_others
                # ACT table load to kernel start, off the sb0 critical chain
                # (separate output tile so the warm-up matmuls don't wait)
                actwarm = qt_pool.tile([128, 8], f16, name="actwarm", tag="actwarm")
                nc.scalar.activation(
                    out=actwarm, in_=warm[:, 496:504], func=Exp
                )
                wps = pq_pool.tile([128, 512], f32, name="wps", tag="wps")
                for _ in range(13):
                    nc.tensor.matmul(
                        wps, lhsT=warm[:, 0:128], rhs=warm[:, :],
                        start=True, stop=True,
                    )
                nc.vector.tensor_copy(out=warm[:, :], in_=wps)

                # ALL startup DMAs go through sync, qt strictly first: any
                # bulk transfer running concurrently with qt (weights issued
                # early from other engines, or wk/kt0 ordered before wq)
                # steals fabric bandwidth and slips qt's landing from 12.5us
                # to >23us — measured twice.  The wq chunks that round-robin
                # onto qt's queues simply wait behind it, which is harmless.
                qt_sb = qt_pool.tile([128, NDC, B], f16)
                qt_v = qT[:, :].rearrange("p (c b) -> p c b", c=NDC)
                for qtr in range(4):
                    cs = slice(qtr * 2, qtr * 2 + 2)
                    nc.sync.dma_start(out=qt_sb[:, cs, :], in_=qt_v[:, cs, :])
                # wk halves + kt0 interleaved into the wq sequence: issued
                # after wq[0-1] they don't starve qt, but land by ~20us so
                # the sb0 kh-projection starts right after qproj (when wk
                # was issued last it landed at 28.6us, the PE idled 7us and
                # HAM re-throttled it to half clock for the next 6.8us)
                wk_v = wkT[:, :].rearrange("p (c n) -> p c n", c=NDC)
                kt0 = kv_pool.tile([128, NDC, SBK], f16, tag="kt", name="kt", bufs=3)
                wq_ts = []

                def _wq_dma(m):
                    wq_t = wq_pool.tile([128, NDC, DC], f16, name="wq_t", bufs=NDC)
                    nc.sync.dma_start(out=wq_t, in_=wqT[m, :, :].rearrange("p (c n) -> p c n", c=NDC))
                    wq_ts.append(wq_t)

                for m in range(2):
                    _wq_dma(m)
                nc.sync.dma_start(out=wk_sb[:, 0:4, :], in_=wk_v[:, 0:4, :])
                for m in range(2, 4):
                    _wq_dma(m)
                nc.sync.dma_start(out=wk_sb[:, 4:8, :], in_=wk_v[:, 4:8, :])
                nc.sync.dma_start(
                    out=kt0, in_=kT[:, :].rearrange("p (s c n) -> p s c n", s=NSB, c=NDC)[:, 0, :, :]
                )
                for m in range(4, NDC):
                    _wq_dma(m)
                for m in range(NDC):
                    pq = pq_pool.tile([128, B], f32, name="pq")
                    for c in range(NDC):
                        nc.tensor.matmul(
                            pq,
                            lhsT=wq_ts[m][:, c, :],
                            rhs=qt_sb[:, c, :],
                            start=(c == 0),
                            stop=(c == NDC - 1),
                        )
                    # scalar (ACT) copy: DVE-free, and the first ACT op also
                    # pulls the exp_and_others table load into the prologue
                    # where it's hidden (saves ~2.7us at first main-loop exp)
                    nc.scalar.copy(out=qh_sb[:, m, :], in_=pq)

            # ---- main loop over key super-blocks ----
            kT_v = kT[:, :].rearrange("p (s c n) -> p s c n", s=NSB, c=NDC)
            vT_v = vT[:, :].rearrange("p (s c n) -> p s c n", s=NSB, c=NDC)
            with (
                tc.tile_pool(name="kh", bufs=2) as kh_pool,
                tc.tile_pool(name="vh", bufs=2) as vh_pool,
                tc.tile_pool(name="pr", bufs=14) as pr_pool,
                tc.tile_pool(name="pp", bufs=2, space="PSUM") as pp_pool,
                tc.tile_pool(name="ps", bufs=2, space="PSUM") as ps_pool,
                tc.tile_pool(name="pc", bufs=2, space="PSUM") as pc_pool,
            ):
                for sb in range(NSB):
                    if sb == 0:
                        kt = kt0
                    else:
                        kt = kv_pool.tile([128, NDC, SBK], f16, tag="kt", name="kt", bufs=3)
                        nc.sync.dma_start(out=kt, in_=kT_v[:, sb, :, :])
                    vt = kv_pool.tile([128, NDC, SBK], f16, tag="vt", name="vt")
                    nc.sync.dma_start(out=vt, in_=vT_v[:, sb, :, :])
                    if sb == 0:
                        nc.sync.dma_start(
                            out=wv_sb,
                            in_=wvT[:, :].rearrange("p (c n) -> p c n", c=NDC),
                        )

                    # kh projection -> kh^T tile [dout(part), m_chunk, keys]
                    khT = kh_pool.tile([128, NDC, SBK], f16, name="khT")
                    for m in range(NDC):
                        pp = pp_pool.tile([128, SBK], f32, tag="pp", name="pp")
                        for c in range(NDC):
                            nc.tensor.matmul(
                                pp,
                                lhsT=wk_sb[:, c, m * DC:(m + 1) * DC],
                                rhs=kt[:, c, :],
                                start=(c == 0),
                                stop=(c == NDC - 1),
                            )
                        # khT evacuation on ACT, not DVE: the DVE queue is
                        # busy with the previous sb's ctx adds, which delayed
                        # these casts ~1us and stalled the scores matmuls
                        nc.scalar.copy(out=khT[:, m, :], in_=pp)

                    # scores + exp: probs^T [keys(part), head, b] in fp16.
                    # Emission order alternates PE row group AND psum bank
                    # (even head -> bank0, odd head -> bank1, ...): row-tiled
                    # matmuls in different row groups execute concurrently on
                    # the PE, and alternating banks keeps every concurrent
                    # pair in different PSUM banks (same-bank pairs share a
                    # row group, which the array serializes) — a same-bank
                    # concurrent write is a fatal PSUM collision.
                    # Groups are emitted TWO at a time with their matmul
                    # pairs interleaved: each PE tiling-mode switch between
                    # row-tiled scores and full-array projection costs a
                    # ~100ns array drain each way, so doubling the burst
                    # halves that tax (16 -> 8 switches per sb).
                    prmap = {}
                    groups = [(hg, kcn) for hg in range(NHG) for kcn in range(NKC)]
                    for j in range(0, len(groups), 2):
                        pss = [
                            ps_pool.tile([128, HG, B], f32, name="ps")
                            for _ in range(2)
                        ]
                        for pair in range(2):
                            for gi in range(2):
                                hg, kcn = groups[j + gi]
                                ps = pss[gi]
                                for h, slot in (
                                    (hg * HG + 2 * pair + 0, pair),
                                    (hg * HG + 2 * pair + 1, 2 + pair),
                                ):
                                    c = h // 2
                                    off = (h % 2) * DH
                                    nc.tensor.matmul(
                                        ps[:, slot, :],
                                        lhsT=khT[off:off + DH, c, kcn * KC:(kcn + 1) * KC],
                                        rhs=qh_sb[off:off + DH, c, :],
                                        start=True,
                                        stop=True,
                                        tile_position=(off, 0),
                                    )
                        for gi in range(2):
                            hg, kcn = groups[j + gi]
                            pr = pr_pool.tile([128, HG, B], f16, name="pr")
                            nc.scalar.activation(
                                out=pr, in_=pss[gi], func=Exp, scale=0.125
                            )
                            for pair in range(2):
                                prmap[(hg * HG + 2 * pair + 0, kcn)] = (pr, pair)
                                prmap[(hg * HG + 2 * pair + 1, kcn)] = (pr, 2 + pair)

                    # vh projection -> vh tile [keys(part), kcn, h, dh+1]
                    # (last column is 1.0 so the ctx matmul also yields the
                    #  softmax denominator on psum row DH).  Emitted after the
                    #  scores so the PE has work while W_v / vt arrive.
                    vh = vh_pool.tile([128, NKC, H, DH + 1], f16, name="vh")
                    nc.gpsimd.memset(vh[:, :, :, DH:DH + 1], 1.0)
                    for kcn in range(NKC):
                        for half in range(2):
                            pp2 = pp_pool.tile([128, SBK], f32, tag="pp", name="pp2")
                            for c in range(NDC):
                                nc.tensor.matmul(
                                    pp2,
                                    lhsT=vt[:, c, kcn * KC:(kcn + 1) * KC],
                                    rhs=wv_sb[:, c, half * 512:(half + 1) * 512],
                                    start=(c == 0),
                                    stop=(c == NDC - 1),
                                )
                            nc.vector.tensor_copy(
                                out=vh[:, kcn, half * 8:(half + 1) * 8, 0:DH],
                                in_=pp2.rearrange("p (h d) -> p h d", h=8),
                            )

                    # context accumulation per head over the super-block
                    for h in range(H):
                        pc = pc_pool.tile([DH + 1, B], f32, name="pc")
                        for kcn in range(NKC):
                            pr, slot = prmap[(h, kcn)]
                            nc.tensor.matmul(
                                pc,
                                lhsT=vh[:, kcn, h, :],
                                rhs=pr[:, slot, :],
                                start=(kcn == 0),
                                stop=(kcn == NKC - 1),
                            )
                        if sb == 0:
                            nc.vector.tensor_copy(out=ctx_sb[:, h, :], in_=pc)
                        else:
                            nc.vector.tensor_add(
                                out=ctx_sb[:, h, :], in0=ctx_sb[:, h, :], in1=pc
                            )
                        if sb == NSB - 1 and h % 2 == 1:
                            # final sb: stream out per head-pair so the last
                            # DMA after the last add is small
                            nc.sync.dma_start(
                                out=out[:, h - 1:h + 1, :],
                                in_=ctx_sb[:, h - 1:h + 1, :],
                            )

            kv_pool.release()
            wq_pool.release()
            qt_pool.release()

    # Run the bacc lowering passes (register allocation, wait splitting via
    # generate_event_semaphores, DCE).  The PJRT execution path serializes
    # nc.m as-is and never calls finalize, so this must happen here.
    nc.compile()
    return nc


def _get_nc():
    if "nc" not in _CACHED:
        _CACHED["nc"] = _build()
    return _CACHED["nc"]


def _swz(wT):
    """[D, X] -> [128, NDC*X] partition-major swizzle (c p) x -> p (c x)."""
    X = wT.shape[1]
    return np.ascontiguousarray(
        wT.reshape(NDC, 128, X).transpose(1, 0, 2).reshape(128, NDC * X)
    )


def _kv_swz(x):
    """[NS, D] shard -> [128, NSB*NDC*SBK] with layout [p, sb, c, n]
    (p = dout % 128 within c-chunk), giving one contiguous 8KB run per
    partition per super-block for single-descriptor-per-partition DMAs."""
    xT = np.ascontiguousarray(x.T).astype(_F16)            # [D, NS]
    x4 = xT.reshape(NDC, 128, NSB, SBK)                    # [c, p, s, n]
    return np.ascontiguousarray(
        x4.transpose(1, 2, 0, 3).reshape(128, NSB * NDC * SBK)
    )


def _prep_inputs(q, k, v, W_q, W_k, W_v):
    """Host-side layout prep: transpose + cast to fp16, shard k/v by N."""
    qT = _swz(np.ascontiguousarray(q.T).astype(_F16))
    wqT_flat = np.ascontiguousarray(W_q.T).astype(_F16)
    # [c, p, m, j] -> [m, p, c, j]
    wqT = np.ascontiguousarray(
        wqT_flat.reshape(NDC, 128, NDC, DC).transpose(2, 1, 0, 3).reshape(NDC, 128, NDC * DC)
    )
    wkT = _swz(np.ascontiguousarray(W_k.T).astype(_F16))
    wvT = _swz(np.ascontiguousarray(W_v.T).astype(_F16))
    in_maps = []
    for core in range(NCORES):
        sl = slice(core * NS, (core + 1) * NS)
        in_maps.append(
            {
                "qT": qT,
                "wqT": wqT,
                "wkT": wkT,
                "wvT": wvT,
                "kT": _kv_swz(k[sl]),
                "vT": _kv_swz(v[sl]),
            }
        )
    return in_maps


def _combine(outs):
    """Sum per-core (num, den) partials and normalize: [65,16,256] x8 -> [B, D]."""
    S = np.zeros((DH + 1, H, B), np.float32)
    for o in outs:
        S += np.asarray(o, np.float32)
    ctx = S[0:DH] / S[DH][None, :, :]          # [dh, h, b]
    return np.ascontiguousarray(ctx.transpose(2, 1, 0).reshape(B, D)).astype(np.float32)


def run(inputs, trace=False, trace_kwargs=None):
    from concourse.bass_utils import run_bass_kernel_spmd

    nc = _get_nc()
    in_maps = _prep_inputs(
        inputs["q"], inputs["k"], inputs["v"],
        inputs["W_q"], inputs["W_k"], inputs["W_v"],
    )
    res = run_bass_kernel_spmd(
        nc,
        in_maps,
        list(range(NCORES)),
        trace=trace,
        **(trace_kwargs or {}),
    )
    out = _combine([res.results[i]["out"] for i in range(NCORES)])
    return out, res


def kernel(**inputs):
    out, _ = run(inputs, trace=False)
    return out

